# revision 19
# baseline (speedup 1.0000x reference)
"""DeViSE forward kernel for Trainium2, 8-core data-parallel over batch.

Model (per reference):
    logits = image @ W_cls.T + b_cls                       [B, C]
    visual = relu(image @ W_vis.T + b_vis)                 [B, H]
    sem    = relu(word_embeddings @ W_sem.T + b_sem)       [C, H]
    scores = visual @ sem.T                                [B, C]

Shapes: B=2048, C=1000, V=2048, H=1024, S=300. All float32.

Sharding: batch split 8 ways (B_loc=256); weights + word embeddings
replicated. No collectives needed (scores are per-sample).

Device layout strategy: every matmul contracts over the partition dim, so
all HBM operands are staged host-side in transposed (contraction-major)
layout: imT=[V, B_loc], WcT=[V, C], WvT=[V, H], WsT=[S, H], weT=[S, C].
Intermediates (visual^T [H,B_loc], sem^T [H,C]) are produced directly in
transposed orientation by choosing matmul output orientation, so the
scores matmul needs no on-chip transposes either.

Matmuls run in float32r (TF32) at full PE rate; accumulation is fp32 in
PSUM, biases/relu applied on ScalarE/VectorE at fp32.

Schedule: the sync HWDGE ring carries the whole input stream in
consumption order (semantic inputs -> imT -> WvT quads -> WcT quads; the
weight streams use 4-k-tile 2MB transfers for HBM efficiency); constants
and output stores ride the scalar HWDGE ring. One 8-bank PSUM pool is
shared by all phases (each accumulation group owns a full bank). The
scores phase is emitted in the middle of the WcT stream loop so its
matmuls and stores fill PE/DMA gaps instead of serializing at the tail.
Measured on 8xNC_v3 (axon): median ~85us, best ~82us; rel err ~1.5e-4.
"""
from contextlib import ExitStack

import numpy as np

import concourse.bass as bass
import concourse.mybir as mybir
import concourse.tile as tile
from concourse import bacc
from concourse.bass_utils import run_bass_kernel_spmd

B, C, V, H, S = 2048, 1000, 2048, 1024, 300
NCORES = 8
BL = B // NCORES          # 256 batch rows per core
KT = V // 128             # 16 k-tiles over visual dim
HT = H // 128             # 8 h-tiles over hidden dim
ST = [(0, 128), (128, 128), (256, 44)]   # k-tiles over semantic dim (300)
CCH = [(0, 512), (512, 488)]             # class chunks (PSUM-bank sized)

F32 = mybir.dt.float32
F32R = mybir.dt.float32r
RELU = mybir.ActivationFunctionType.Relu

_CACHE = {}


def _build(scores_at_kk=None, wvp_bufs=4, wcp_bufs=6, sem_split=False,
           warmup=0, asserts=True, im_engine="sync", kgrp=2,
           split_head=False):
    nc = bacc.Bacc("TRN2", target_bir_lowering=False, debug=False,
                   enable_asserts=asserts)

    imT = nc.dram_tensor("imT", [V, BL], F32R, kind="ExternalInput")
    WcT = nc.dram_tensor("WcT", [V, C], F32R, kind="ExternalInput")
    WvT = nc.dram_tensor("WvT", [V, H], F32R, kind="ExternalInput")
    WsT = nc.dram_tensor("WsT", [S, H], F32R, kind="ExternalInput")
    weT = nc.dram_tensor("weT", [S, C], F32R, kind="ExternalInput")
    bvis = nc.dram_tensor("bvis", [128, HT], F32, kind="ExternalInput")
    bsem = nc.dram_tensor("bsem", [128, HT], F32, kind="ExternalInput")
    bcls = nc.dram_tensor("bcls", [1, C], F32, kind="ExternalInput")
    logits_o = nc.dram_tensor("logits_o", [BL, C], F32, kind="ExternalOutput")
    scores_o = nc.dram_tensor("scores_o", [BL, C], F32, kind="ExternalOutput")

    with tile.TileContext(nc) as tc, ExitStack() as ctx:
        cst = ctx.enter_context(tc.tile_pool(name="cst", bufs=1))
        semin = ctx.enter_context(tc.tile_pool(name="semin", bufs=1))
        wvp = ctx.enter_context(tc.tile_pool(name="wvp", bufs=wvp_bufs))
        wcp = ctx.enter_context(tc.tile_pool(name="wcp", bufs=wcp_bufs))
        semp = ctx.enter_context(tc.tile_pool(name="semp", bufs=HT))
        visp = ctx.enter_context(tc.tile_pool(name="visp", bufs=HT))
        ostg = ctx.enter_context(tc.tile_pool(name="ostg", bufs=4))
        ps = ctx.enter_context(tc.tile_pool(name="ps", bufs=8, space="PSUM"))

        if warmup:
            BF16 = mybir.dt.bfloat16
            wu_l = cst.tile([128, 128], BF16, tag="wu_l")
            nc.vector.memset(wu_l[:], 0.0)
            wu_r = cst.tile([128, 512], BF16, tag="wu_r")
            nc.vector.memset(wu_r[:], 0.0)
            wu_p = [ps.tile([128, 512], F32, tag="ps", name=f"wu_p{i}")
                    for i in range(2)]
            for i in range(warmup):
                nc.tensor.matmul(wu_p[i % 2][:], wu_l[:], wu_r[:],
                                 start=True, stop=True)

        # ---- input stream, in consumption order, on the sync ring ----
        ws = []
        we = []
        if split_head:
            # small head chunks first so the first semantic group can start
            # as early as possible; remainders follow.
            for i, (s0, sn) in enumerate(ST):
                t = semin.tile([128, H], F32R, tag=f"ws{i}", name=f"ws{i}")
                nc.sync.dma_start(t[:sn, 0:128], WsT[s0:s0 + sn, 0:128])
                ws.append(t)
                t = semin.tile([128, C], F32R, tag=f"we{i}", name=f"we{i}")
                nc.sync.dma_start(t[:sn, 0:512], weT[s0:s0 + sn, 0:512])
                we.append(t)
            for i, (s0, sn) in enumerate(ST):
                nc.sync.dma_start(ws[i][:sn, 128:H], WsT[s0:s0 + sn, 128:H])
                nc.sync.dma_start(we[i][:sn, 512:C], weT[s0:s0 + sn, 512:C])
        else:
            for i, (s0, sn) in enumerate(ST):
                t = semin.tile([128, H], F32R, tag=f"ws{i}", name=f"ws{i}")
                nc.sync.dma_start(t[:sn, :], WsT[s0:s0 + sn, :])
                ws.append(t)
                t = semin.tile([128, C], F32R, tag=f"we{i}", name=f"we{i}")
                nc.sync.dma_start(t[:sn, :], weT[s0:s0 + sn, :])
                we.append(t)

        im = cst.tile([128, KT, BL], F32R, tag="im")
        im_eng = {"sync": nc.sync, "gpsimd": nc.gpsimd}[im_engine]
        im_eng.dma_start(im[:], imT.ap().rearrange("(a p) n -> p a n", p=128))

        # constants on the scalar ring (don't block the input stream)
        bv = cst.tile([128, HT], F32, tag="bv")
        nc.scalar.dma_start(bv[:], bvis[:, :])
        bs = cst.tile([128, HT], F32, tag="bs")
        nc.scalar.dma_start(bs[:], bsem[:, :])
        bc_row = cst.tile([1, C], F32, tag="bc_row")
        nc.scalar.dma_start(bc_row[:], bcls[:, :])
        bc = cst.tile([128, C], F32, tag="bc")
        nc.gpsimd.partition_broadcast(bc[:], bc_row[:1, :])

        # ---- semantic path: semT[h] = relu(WsT.T @ weT + b_sem)  [H, C] ----
        semT = [semp.tile([128, C], F32R, tag="semT", name=f"semT{h}")
                for h in range(HT)]
        for h in range(HT):
            for c0, cn in CCH:
                p = ps.tile([128, 512], F32, tag="ps")
                for i, (s0, sn) in enumerate(ST):
                    nc.tensor.matmul(
                        p[:, :cn],
                        ws[i][:sn, h * 128:(h + 1) * 128],
                        we[i][:sn, c0:c0 + cn],
                        start=(i == 0), stop=(i == len(ST) - 1),
                    )
                if sem_split and (h + (c0 > 0)) % 2 == 1:
                    nc.vector.tensor_scalar(
                        semT[h][:, c0:c0 + cn], p[:, :cn],
                        bs[:, h:h + 1], 0.0,
                        mybir.AluOpType.add, mybir.AluOpType.max)
                else:
                    nc.scalar.activation(semT[h][:, c0:c0 + cn], p[:, :cn],
                                         RELU, bias=bs[:, h:h + 1])

        # ---- visual path: visT[h] = relu(WvT.T @ imT + b_vis)  [H, B_loc]
        psv = [ps.tile([128, BL], F32, tag="ps", name=f"psv{h}")
               for h in range(HT)]
        for kk in range(KT // kgrp):
            wv2 = wvp.tile([128, kgrp, H], F32R, tag="wv")
            nc.sync.dma_start(
                wv2[:], WvT[kk * kgrp * 128:(kk + 1) * kgrp * 128, :]
                .rearrange("(a p) n -> p a n", p=128))
            for k2 in range(kgrp):
                k = kk * kgrp + k2
                for h in range(HT):
                    nc.tensor.matmul(
                        psv[h][:, :],
                        wv2[:, k2, h * 128:(h + 1) * 128],
                        im[:, k, :],
                        start=(k == 0), stop=(k == KT - 1),
                    )
        visT = [visp.tile([128, BL], F32R, tag="visT", name=f"visT{h}")
                for h in range(HT)]
        for h in range(HT):
            nc.scalar.activation(visT[h][:], psv[h][:], RELU,
                                 bias=bv[:, h:h + 1])

        def scores_phase():
            # scores: visT.T @ semT  [B_loc, C]
            for b in range(BL // 128):
                for c0, cn in CCH:
                    p = ps.tile([128, 512], F32, tag="ps", name=f"pss{b}_{c0}")
                    for h in range(HT):
                        nc.tensor.matmul(
                            p[:, :cn],
                            visT[h][:, b * 128:(b + 1) * 128],
                            semT[h][:, c0:c0 + cn],
                            start=(h == 0), stop=(h == HT - 1),
                        )
                    o = ostg.tile([128, 512], F32, tag="ostg",
                                  name=f"so{b}_{c0}")
                    nc.vector.tensor_copy(o[:, :cn], p[:, :cn])
                    nc.scalar.dma_start(
                        scores_o[b * 128:(b + 1) * 128, c0:c0 + cn], o[:, :cn])

        # ---- logits: imT.T @ WcT + b_cls  [B_loc, C] ----
        psl = {}
        for b in range(BL // 128):
            for ci, (c0, cn) in enumerate(CCH):
                psl[(b, ci)] = ps.tile([128, 512], F32, tag="ps",
                                       name=f"psl{b}_{ci}")
        for kk in range(KT // kgrp):
            wc2 = wcp.tile([128, kgrp, C], F32R, tag="wc")
            nc.sync.dma_start(
                wc2[:], WcT[kk * kgrp * 128:(kk + 1) * kgrp * 128, :]
                .rearrange("(a p) n -> p a n", p=128))
            for k2 in range(kgrp):
                k = kk * kgrp + k2
                for b in range(BL // 128):
                    for ci, (c0, cn) in enumerate(CCH):
                        nc.tensor.matmul(
                            psl[(b, ci)][:, :cn],
                            im[:, k, b * 128:(b + 1) * 128],
                            wc2[:, k2, c0:c0 + cn],
                            start=(k == 0), stop=(k == KT - 1),
                        )
            if kk == scores_at_kk:
                scores_phase()
        for b in range(BL // 128):
            for ci, (c0, cn) in enumerate(CCH):
                o = ostg.tile([128, 512], F32, tag="ostg")
                nc.vector.tensor_add(o[:, :cn], psl[(b, ci)][:, :cn],
                                     bc[:, c0:c0 + cn])
                nc.scalar.dma_start(
                    logits_o[b * 128:(b + 1) * 128, c0:c0 + cn], o[:, :cn])
        if scores_at_kk is None:
            scores_phase()

    nc.compile()
    return nc


def _get_nc():
    if "nc" not in _CACHE:
        _CACHE["nc"] = _build(kgrp=4, wvp_bufs=3, wcp_bufs=3, scores_at_kk=2)
    return _CACHE["nc"]


def _in_maps(image, word_embeddings, W_cls, b_cls, W_vis, b_vis, W_sem, b_sem):
    f32 = np.float32
    imT = np.ascontiguousarray(np.asarray(image, dtype=f32).T)          # [V, B]
    WcT = np.ascontiguousarray(np.asarray(W_cls, dtype=f32).T)          # [V, C]
    WvT = np.ascontiguousarray(np.asarray(W_vis, dtype=f32).T)          # [V, H]
    WsT = np.ascontiguousarray(np.asarray(W_sem, dtype=f32).T)          # [S, H]
    weT = np.ascontiguousarray(np.asarray(word_embeddings, dtype=f32).T)  # [S, C]
    bv = np.ascontiguousarray(np.asarray(b_vis, dtype=f32).reshape(HT, 128).T)
    bs = np.ascontiguousarray(np.asarray(b_sem, dtype=f32).reshape(HT, 128).T)
    bc = np.ascontiguousarray(np.asarray(b_cls, dtype=f32).reshape(1, C))
    maps = []
    for c in range(NCORES):
        maps.append({
            "imT": np.ascontiguousarray(imT[:, c * BL:(c + 1) * BL]),
            "WcT": WcT, "WvT": WvT, "WsT": WsT, "weT": weT,
            "bvis": bv, "bsem": bs, "bcls": bc,
        })
    return maps


def run(trace=False, tmpdir=None, **inputs):
    nc = _get_nc()
    maps = _in_maps(**inputs)
    res = run_bass_kernel_spmd(nc, maps, list(range(NCORES)), trace=trace,
                               tmpdir=tmpdir)
    logits = np.concatenate(
        [res.results[c]["logits_o"] for c in range(NCORES)], axis=0)
    scores = np.concatenate(
        [res.results[c]["scores_o"] for c in range(NCORES)], axis=0)
    return (logits, scores), res


def kernel(**inputs):
    out, _ = run(trace=False, **inputs)
    return out


# revision 23
# speedup vs baseline: 1.0600x; 1.0600x over previous
"""DeViSE forward kernel for Trainium2, 8-core data-parallel over batch.

Model (per reference):
    logits = image @ W_cls.T + b_cls                       [B, C]
    visual = relu(image @ W_vis.T + b_vis)                 [B, H]
    sem    = relu(word_embeddings @ W_sem.T + b_sem)       [C, H]
    scores = visual @ sem.T                                [B, C]

Shapes: B=2048, C=1000, V=2048, H=1024, S=300. All float32.

Sharding: batch split 8 ways (B_loc=256); weights + word embeddings
replicated. No collectives needed (scores are per-sample).

Device layout strategy: every matmul contracts over the partition dim, so
all HBM operands are staged host-side in transposed (contraction-major)
layout: imT=[V, B_loc], WcT=[V, C], WvT=[V, H], WsT=[S, H], weT=[S, C].
Intermediates (visual^T [H,B_loc], sem^T [H,C]) are produced directly in
transposed orientation by choosing matmul output orientation, so the
scores matmul needs no on-chip transposes either.

Matmuls run in float32r (TF32) at full PE rate; accumulation is fp32 in
PSUM, biases/relu applied on ScalarE/VectorE at fp32.

Schedule: the sync HWDGE ring carries the whole input stream in
consumption order: [W_sem|word_emb] concatenated 1MB transfers (host
staging packs them so the first semantic group starts one transfer in),
then imT, then WvT 4-k-tile 2MB quads, then the WcT stream tapered
(2MB,2MB,2MB,1MB,0.5MB,0.5MB) so the final logits matmuls pipeline
against the stream tail. Constants and output stores (one merged 500KB
store per 128-row block) ride the scalar HWDGE ring. One 8-bank PSUM
pool is shared by all phases (each accumulation group owns a full bank).
The scores phase is emitted mid-WcT-stream so its matmuls and stores
fill PE/DMA gaps instead of serializing at the tail.
Measured on 8xNC_v3 (axon): median ~83us, best ~82us; rel err ~1.5e-4.
(Run-to-run HAM/clock-gate variance is +-6%; medians of >=5 runs.)
"""
from contextlib import ExitStack

import numpy as np

import concourse.bass as bass
import concourse.mybir as mybir
import concourse.tile as tile
from concourse import bacc
from concourse.bass_utils import run_bass_kernel_spmd

B, C, V, H, S = 2048, 1000, 2048, 1024, 300
NCORES = 8
BL = B // NCORES          # 256 batch rows per core
KT = V // 128             # 16 k-tiles over visual dim
HT = H // 128             # 8 h-tiles over hidden dim
ST = [(0, 128), (128, 128), (256, 44)]   # k-tiles over semantic dim (300)
CCH = [(0, 512), (512, 488)]             # class chunks (PSUM-bank sized)

F32 = mybir.dt.float32
F32R = mybir.dt.float32r
RELU = mybir.ActivationFunctionType.Relu

_CACHE = {}


def _build(scores_at_kk=None, wvp_bufs=4, wcp_bufs=6, sem_split=False,
           warmup=0, asserts=True, im_engine="sync", kgrp=2,
           split_head=False):
    nc = bacc.Bacc("TRN2", target_bir_lowering=False, debug=False,
                   enable_asserts=asserts)

    imT = nc.dram_tensor("imT", [V, BL], F32R, kind="ExternalInput")
    WcT = nc.dram_tensor("WcT", [V, C], F32R, kind="ExternalInput")
    WvT = nc.dram_tensor("WvT", [V, H], F32R, kind="ExternalInput")
    semcat = nc.dram_tensor("semcat", [S, H + C], F32R, kind="ExternalInput")
    bvis = nc.dram_tensor("bvis", [128, HT], F32, kind="ExternalInput")
    bsem = nc.dram_tensor("bsem", [128, HT], F32, kind="ExternalInput")
    bcls = nc.dram_tensor("bcls", [1, C], F32, kind="ExternalInput")
    logits_o = nc.dram_tensor("logits_o", [BL, C], F32, kind="ExternalOutput")
    scores_o = nc.dram_tensor("scores_o", [BL, C], F32, kind="ExternalOutput")

    with tile.TileContext(nc) as tc, ExitStack() as ctx:
        cst = ctx.enter_context(tc.tile_pool(name="cst", bufs=1))
        semin = ctx.enter_context(tc.tile_pool(name="semin", bufs=1))
        wvp = ctx.enter_context(tc.tile_pool(name="wvp", bufs=wvp_bufs))
        wcp = ctx.enter_context(tc.tile_pool(name="wcp", bufs=wcp_bufs))
        semp = ctx.enter_context(tc.tile_pool(name="semp", bufs=HT))
        visp = ctx.enter_context(tc.tile_pool(name="visp", bufs=HT))
        ostg = ctx.enter_context(tc.tile_pool(name="ostg", bufs=4))
        ps = ctx.enter_context(tc.tile_pool(name="ps", bufs=8, space="PSUM"))

        if warmup:
            BF16 = mybir.dt.bfloat16
            wu_l = cst.tile([128, 128], BF16, tag="wu_l")
            nc.vector.memset(wu_l[:], 0.0)
            wu_r = cst.tile([128, 512], BF16, tag="wu_r")
            nc.vector.memset(wu_r[:], 0.0)
            wu_p = [ps.tile([128, 512], F32, tag="ps", name=f"wu_p{i}")
                    for i in range(2)]
            for i in range(warmup):
                nc.tensor.matmul(wu_p[i % 2][:], wu_l[:], wu_r[:],
                                 start=True, stop=True)

        # ---- input stream, in consumption order, on the sync ring ----
        # one 1MB transfer per semantic k-tile carrying [WsT | weT] columns
        ws = []
        we = []
        for i, (s0, sn) in enumerate(ST):
            t = semin.tile([128, H + C], F32R, tag=f"sc{i}", name=f"sc{i}")
            nc.sync.dma_start(t[:sn, :], semcat[s0:s0 + sn, :])
            ws.append(t[:, 0:H])
            we.append(t[:, H:H + C])

        im = cst.tile([128, KT, BL], F32R, tag="im")
        im_eng = {"sync": nc.sync, "gpsimd": nc.gpsimd,
                  "nobcast": nc.sync}[im_engine]
        im_eng.dma_start(im[:], imT.ap().rearrange("(a p) n -> p a n", p=128))

        # constants on the scalar ring (don't block the input stream)
        bv = cst.tile([128, HT], F32, tag="bv")
        nc.scalar.dma_start(bv[:], bvis[:, :])
        bs = cst.tile([128, HT], F32, tag="bs")
        nc.scalar.dma_start(bs[:], bsem[:, :])
        bc_row = cst.tile([1, C], F32, tag="bc_row")
        nc.scalar.dma_start(bc_row[:], bcls[:, :])
        bc = cst.tile([128, C], F32, tag="bc")
        if im_engine == "nobcast":
            nc.scalar.dma_start(bc[:], bcls[0:1, :].broadcast_to((128, C)))
        else:
            nc.gpsimd.partition_broadcast(bc[:], bc_row[:1, :])

        # ---- semantic path: semT[h] = relu(WsT.T @ weT + b_sem)  [H, C] ----
        semT = [semp.tile([128, C], F32R, tag="semT", name=f"semT{h}")
                for h in range(HT)]
        for h in range(HT):
            for c0, cn in CCH:
                p = ps.tile([128, 512], F32, tag="ps")
                for i, (s0, sn) in enumerate(ST):
                    nc.tensor.matmul(
                        p[:, :cn],
                        ws[i][:sn, h * 128:(h + 1) * 128],
                        we[i][:sn, c0:c0 + cn],
                        start=(i == 0), stop=(i == len(ST) - 1),
                    )
                if sem_split and (h + (c0 > 0)) % 2 == 1:
                    nc.vector.tensor_scalar(
                        semT[h][:, c0:c0 + cn], p[:, :cn],
                        bs[:, h:h + 1], 0.0,
                        mybir.AluOpType.add, mybir.AluOpType.max)
                else:
                    nc.scalar.activation(semT[h][:, c0:c0 + cn], p[:, :cn],
                                         RELU, bias=bs[:, h:h + 1])

        # ---- visual path: visT[h] = relu(WvT.T @ imT + b_vis)  [H, B_loc]
        psv = [ps.tile([128, BL], F32, tag="ps", name=f"psv{h}")
               for h in range(HT)]
        for kk in range(KT // kgrp):
            wv2 = wvp.tile([128, kgrp, H], F32R, tag="wv")
            nc.sync.dma_start(
                wv2[:], WvT[kk * kgrp * 128:(kk + 1) * kgrp * 128, :]
                .rearrange("(a p) n -> p a n", p=128))
            for k2 in range(kgrp):
                k = kk * kgrp + k2
                for h in range(HT):
                    nc.tensor.matmul(
                        psv[h][:, :],
                        wv2[:, k2, h * 128:(h + 1) * 128],
                        im[:, k, :],
                        start=(k == 0), stop=(k == KT - 1),
                    )
        visT = [visp.tile([128, BL], F32R, tag="visT", name=f"visT{h}")
                for h in range(HT)]
        for h in range(HT):
            nc.scalar.activation(visT[h][:], psv[h][:], RELU,
                                 bias=bv[:, h:h + 1])

        def scores_phase():
            # scores: visT.T @ semT  [B_loc, C]
            for b in range(BL // 128):
                o = ostg.tile([128, H + 0], F32, tag="ostgw", name=f"so{b}")
                for c0, cn in CCH:
                    p = ps.tile([128, 512], F32, tag="ps", name=f"pss{b}_{c0}")
                    for h in range(HT):
                        nc.tensor.matmul(
                            p[:, :cn],
                            visT[h][:, b * 128:(b + 1) * 128],
                            semT[h][:, c0:c0 + cn],
                            start=(h == 0), stop=(h == HT - 1),
                        )
                    nc.vector.tensor_copy(o[:, c0:c0 + cn], p[:, :cn])
                nc.scalar.dma_start(
                    scores_o[b * 128:(b + 1) * 128, :], o[:, :C])

        # ---- logits: imT.T @ WcT + b_cls  [B_loc, C] ----
        psl = {}
        for b in range(BL // 128):
            for ci, (c0, cn) in enumerate(CCH):
                psl[(b, ci)] = ps.tile([128, 512], F32, tag="ps",
                                       name=f"psl{b}_{ci}")
        wc_chunks = [(0, 4), (4, 4), (8, 4), (12, 2), (14, 1), (15, 1)]
        for kki, (k0, nk) in enumerate(wc_chunks):
            wc2 = wcp.tile([128, nk, C], F32R, tag="wc")
            nc.sync.dma_start(
                wc2[:], WcT[k0 * 128:(k0 + nk) * 128, :]
                .rearrange("(a p) n -> p a n", p=128))
            for k2 in range(nk):
                k = k0 + k2
                for b in range(BL // 128):
                    for ci, (c0, cn) in enumerate(CCH):
                        nc.tensor.matmul(
                            psl[(b, ci)][:, :cn],
                            im[:, k, b * 128:(b + 1) * 128],
                            wc2[:, k2, c0:c0 + cn],
                            start=(k == 0), stop=(k == KT - 1),
                        )
            if kki == scores_at_kk:
                scores_phase()
        for b in range(BL // 128):
            o = ostg.tile([128, H + 0], F32, tag="ostgw", name=f"lo{b}")
            for ci, (c0, cn) in enumerate(CCH):
                nc.vector.tensor_add(o[:, c0:c0 + cn], psl[(b, ci)][:, :cn],
                                     bc[:, c0:c0 + cn])
            nc.scalar.dma_start(
                logits_o[b * 128:(b + 1) * 128, :], o[:, :C])
        if scores_at_kk is None:
            scores_phase()

    nc.compile()
    return nc


def _get_nc():
    if "nc" not in _CACHE:
        _CACHE["nc"] = _build(kgrp=4, wvp_bufs=3, wcp_bufs=3, scores_at_kk=2)
    return _CACHE["nc"]


def _in_maps(image, word_embeddings, W_cls, b_cls, W_vis, b_vis, W_sem, b_sem):
    f32 = np.float32
    imT = np.ascontiguousarray(np.asarray(image, dtype=f32).T)          # [V, B]
    WcT = np.ascontiguousarray(np.asarray(W_cls, dtype=f32).T)          # [V, C]
    WvT = np.ascontiguousarray(np.asarray(W_vis, dtype=f32).T)          # [V, H]
    semcat = np.ascontiguousarray(np.concatenate(
        [np.asarray(W_sem, dtype=f32).T,
         np.asarray(word_embeddings, dtype=f32).T], axis=1))  # [S, H+C]
    bv = np.ascontiguousarray(np.asarray(b_vis, dtype=f32).reshape(HT, 128).T)
    bs = np.ascontiguousarray(np.asarray(b_sem, dtype=f32).reshape(HT, 128).T)
    bc = np.ascontiguousarray(np.asarray(b_cls, dtype=f32).reshape(1, C))
    maps = []
    for c in range(NCORES):
        maps.append({
            "imT": np.ascontiguousarray(imT[:, c * BL:(c + 1) * BL]),
            "WcT": WcT, "WvT": WvT, "semcat": semcat,
            "bvis": bv, "bsem": bs, "bcls": bc,
        })
    return maps


def run(trace=False, tmpdir=None, **inputs):
    nc = _get_nc()
    maps = _in_maps(**inputs)
    res = run_bass_kernel_spmd(nc, maps, list(range(NCORES)), trace=trace,
                               tmpdir=tmpdir)
    logits = np.concatenate(
        [res.results[c]["logits_o"] for c in range(NCORES)], axis=0)
    scores = np.concatenate(
        [res.results[c]["scores_o"] for c in range(NCORES)], axis=0)
    return (logits, scores), res


def kernel(**inputs):
    out, _ = run(trace=False, **inputs)
    return out
